# revision 1
# baseline (speedup 1.0000x reference)
"""Llama3 GQA decode attention (B=8, q_len=1, past=4096) on 8 TRN2 cores.

Sharding: tensor-parallel over heads. Core c owns q-heads [4c, 4c+4) and
kv-head c: Wq/Wk/Wv output-dim sharded, Wo input-dim sharded, KV cache
sharded by kv head. Each core computes a partial o_proj output [8, 4096];
the host sum over cores is the all-reduce.

Device-side layouts (host prepares, data movement only):
  xT    [4096, 8]      x transposed
  wqkvT [4096, 768]    concat(Wq_c, Wk_c, Wv_c).T  (in-dim major)
  woT   [512, 4096]    Wo[:, 512c:512c+512].T
  kT    [8, 128, 4096] past_k[:, c] with d on partitions (pre-transposed)
  v     [8, 4096, 128] past_v[:, c] natural
All matmuls contract over the partition dim; with these layouts no large
transpose is ever needed on device.
"""

import sys

sys.path.insert(0, "/opt/trn_rl_repo")

import numpy as np

import concourse.bacc as bacc
import concourse.tile as tile
from concourse import mybir
from concourse.bass_utils import run_bass_kernel_spmd

B = 8            # batch
NH = 32          # query heads total
NKV = 8          # kv heads total
D = 128          # head dim
HID = 4096       # hidden
S = 4096         # past length
NCORES = 8
HQ = NH // NCORES          # 4 query heads per core
QKV = HQ * D + 2 * D       # 768 projection outputs per core
T = S // 128               # 32 seq tiles
ROPE_THETA = 500000.0

F32 = mybir.dt.float32
EXP = mybir.ActivationFunctionType.Exp

_CACHE = {}


def _build_module():
    nc = bacc.Bacc()
    xT = nc.declare_dram_parameter("xT", [HID, B], F32, isOutput=False)
    wqkvT = nc.declare_dram_parameter("wqkvT", [HID, QKV], F32, isOutput=False)
    woT = nc.declare_dram_parameter("woT", [HQ * D, HID], F32, isOutput=False)
    kT = nc.declare_dram_parameter("kT", [B, D, S], F32, isOutput=False)
    v = nc.declare_dram_parameter("v", [B, S, D], F32, isOutput=False)
    ropes = nc.declare_dram_parameter("ropes", [D, 4], F32, isOutput=False)
    ones = nc.declare_dram_parameter("ones", [D, 1], F32, isOutput=False)
    onesr = nc.declare_dram_parameter("onesr", [1, D], F32, isOutput=False)
    o = nc.declare_dram_parameter("o", [B, HID], F32, isOutput=True)

    with tile.TileContext(nc) as tc:
        from contextlib import ExitStack

        with ExitStack() as ctx:
            consts = ctx.enter_context(tc.tile_pool(name="consts", bufs=1))
            w_pool = ctx.enter_context(tc.tile_pool(name="w", bufs=3))
            kv_pool = ctx.enter_context(tc.tile_pool(name="kv", bufs=3))
            exp_pool = ctx.enter_context(tc.tile_pool(name="exp", bufs=2))
            small = ctx.enter_context(tc.tile_pool(name="small", bufs=2))
            wo_pool = ctx.enter_context(tc.tile_pool(name="wo", bufs=4))
            osb_pool = ctx.enter_context(tc.tile_pool(name="osb", bufs=4))

            # ---- constants / persistent SBUF ----
            x_sb = consts.tile([128, T, B], F32)
            nc.sync.dma_start(out=x_sb, in_=xT[:, :].rearrange("(t p) b -> p t b", p=128))
            ropes_sb = consts.tile([D, 4], F32)
            nc.sync.dma_start(out=ropes_sb, in_=ropes[:, :])
            ones_sb = consts.tile([D, 1], F32)
            nc.sync.dma_start(out=ones_sb, in_=ones[:, :])
            onesr_sb = consts.tile([1, D], F32)
            nc.sync.dma_start(out=onesr_sb, in_=onesr[:, :])

            qT_sb = consts.tile([D, HQ, B], F32)     # roped qT, scaled by 1/sqrt(D)
            kTn_sb = consts.tile([D, B], F32)        # roped new-k (kT layout)
            vT_sb = consts.tile([D, B], F32)         # new v, transposed layout
            oT_sb = consts.tile([D, HQ, B], F32)     # normalized attn out, oT layout

            qcos = ropes_sb[:, 0:1]
            qsin = ropes_sb[:, 1:2]
            kcos = ropes_sb[:, 2:3]
            ksin = ropes_sb[:, 3:4]

            # ---- phase A: QKV projections (weights stationary) ----
            with tc.tile_pool(name="psA", bufs=6, space="PSUM") as psA:
                pj = [psA.tile([D, B], F32, tag="pj", name=f"pj{j}", bufs=6)
                      for j in range(HQ + 2)]
                for t in range(T):
                    w_sb = w_pool.tile([128, QKV], F32)
                    nc.sync.dma_start(out=w_sb, in_=wqkvT[t * 128:(t + 1) * 128, :])
                    for j in range(HQ + 2):
                        nc.tensor.matmul(
                            pj[j], w_sb[:, j * D:(j + 1) * D], x_sb[:, t, :],
                            start=(t == 0), stop=(t == T - 1),
                        )

                # RoPE on qT (per-partition cos/sin) + scale baked into consts
                for h in range(HQ):
                    shuf = small.tile([D, B], F32, tag="shuf")
                    nc.vector.tensor_copy(shuf[0:64, :], pj[h][64:128, :])
                    nc.vector.tensor_copy(shuf[64:128, :], pj[h][0:64, :])
                    nc.vector.tensor_scalar_mul(shuf, shuf, qsin)
                    nc.vector.scalar_tensor_tensor(
                        out=qT_sb[:, h, :], in0=pj[h], scalar=qcos,
                        in1=shuf, op0=mybir.AluOpType.mult, op1=mybir.AluOpType.add,
                    )
                # RoPE on new k
                shufk = small.tile([D, B], F32, tag="shuf")
                nc.vector.tensor_copy(shufk[0:64, :], pj[HQ][64:128, :])
                nc.vector.tensor_copy(shufk[64:128, :], pj[HQ][0:64, :])
                nc.vector.tensor_scalar_mul(shufk, shufk, ksin)
                nc.vector.scalar_tensor_tensor(
                    out=kTn_sb, in0=pj[HQ], scalar=kcos,
                    in1=shufk, op0=mybir.AluOpType.mult, op1=mybir.AluOpType.add,
                )
                # new v stays in transposed layout [d, b]
                nc.vector.tensor_copy(vT_sb, pj[HQ + 1])

            # ---- phase B: attention per batch ----
            with tc.tile_pool(name="psST", bufs=2, space="PSUM") as psST, \
                 tc.tile_pool(name="psOT", bufs=2, space="PSUM") as psOT, \
                 tc.tile_pool(name="psSL", bufs=1, space="PSUM") as psSL, \
                 tc.tile_pool(name="psZ", bufs=1, space="PSUM") as psZ, \
                 tc.tile_pool(name="psZB", bufs=1, space="PSUM") as psZB:
                for b in range(B):
                    # split each 2MiB cache load into 4 chunks so multiple DMA
                    # queues run in parallel (single-queue BW < HBM peak)
                    kt_b = kv_pool.tile([128, T, D], F32, tag="kt")
                    kT_v = kT[b].rearrange("p (t s) -> p t s", s=128)
                    v_b = kv_pool.tile([128, T, D], F32, tag="v")
                    v_v = v[b].rearrange("(t p) d -> p t d", p=128)
                    CH = T // 4
                    for ci in range(4):
                        sl_ = slice(ci * CH, (ci + 1) * CH)
                        nc.sync.dma_start(out=kt_b[:, sl_, :], in_=kT_v[:, sl_, :])
                        nc.sync.dma_start(out=v_b[:, sl_, :], in_=v_v[:, sl_, :])

                    exp_sb = exp_pool.tile([128, T, HQ], F32)
                    oT_ps = psOT.tile([D, HQ], F32)
                    for t in range(T):
                        st = psST.tile([128, HQ], F32)
                        nc.tensor.matmul(st, kt_b[:, t, :], qT_sb[:, :, b],
                                         start=True, stop=True)
                        nc.scalar.activation(out=exp_sb[:, t, :], in_=st, func=EXP)
                        nc.tensor.matmul(oT_ps, v_b[:, t, :], exp_sb[:, t, :],
                                         start=(t == 0), stop=(t == T - 1))
                    # current position (no mask needed: q_pos >= all k_pos)
                    sl = psSL.tile([1, HQ], F32)
                    nc.tensor.matmul(sl, kTn_sb[:, b:b + 1], qT_sb[:, :, b],
                                     start=True, stop=True)
                    expl = small.tile([1, HQ], F32, tag="expl")
                    nc.scalar.activation(out=expl, in_=sl, func=EXP)
                    # rank-1 update v_new[d] * expl[h], via broadcast matmul + DVE
                    eb_ps = psZB.tile([128, HQ], F32, tag="eb")
                    nc.tensor.matmul(eb_ps, onesr_sb, expl, start=True, stop=True)
                    vl_sb = small.tile([128, HQ], F32, tag="vl")
                    nc.vector.tensor_scalar_mul(vl_sb, eb_ps, vT_sb[:, b:b + 1])

                    # softmax denominator Z = sum(exp)  (partition+tile sum)
                    zpart = small.tile([128, HQ], F32, tag="zpart")
                    nc.vector.reduce_sum(
                        out=zpart, in_=exp_sb.rearrange("p t h -> p h t"),
                        axis=mybir.AxisListType.X)
                    z_ps = psZ.tile([1, HQ], F32)
                    nc.tensor.matmul(z_ps, ones_sb, zpart, start=True, stop=False)
                    nc.tensor.matmul(z_ps, ones_sb[0:1, :], expl,
                                     start=False, stop=True)
                    rz = small.tile([1, HQ], F32, tag="rz")
                    nc.vector.reciprocal(rz, z_ps)
                    zb_ps = psZB.tile([128, HQ], F32, tag="zb")
                    nc.tensor.matmul(zb_ps, onesr_sb, rz, start=True, stop=True)
                    zb_sb = small.tile([128, HQ], F32, tag="zbs")
                    nc.vector.tensor_copy(zb_sb, zb_ps)
                    # (cache PV + new-token term), normalize, scatter to [d, h, b]
                    s1_sb = small.tile([128, HQ], F32, tag="s1")
                    nc.vector.tensor_add(s1_sb, oT_ps, vl_sb)
                    nc.vector.tensor_mul(oT_sb[:, :, b], s1_sb, zb_sb)

            # ---- phase C: o_proj partial ----
            with tc.tile_pool(name="psO", bufs=4, space="PSUM") as psO:
                wo_sb = []
                for h in range(HQ):
                    w = wo_pool.tile([D, HID], F32)
                    nc.sync.dma_start(out=w, in_=woT[h * D:(h + 1) * D, :])
                    wo_sb.append(w)
                for n in range(HID // 512):
                    o_ps = psO.tile([B, 512], F32)
                    for h in range(HQ):
                        nc.tensor.matmul(
                            o_ps, oT_sb[:, h, :], wo_sb[h][:, n * 512:(n + 1) * 512],
                            start=(h == 0), stop=(h == HQ - 1))
                    o_sb = osb_pool.tile([B, 512], F32)
                    nc.vector.tensor_copy(o_sb, o_ps)
                    nc.sync.dma_start(out=o[:, n * 512:(n + 1) * 512], in_=o_sb)

    nc.compile()
    return nc


def _host_constants():
    inv = ROPE_THETA ** (-np.arange(0, 64, dtype=np.float64) * 2.0 / D)
    ang = float(S) * inv
    cos = np.cos(np.concatenate([ang, ang])).astype(np.float64)
    sin = np.sin(np.concatenate([ang, ang])).astype(np.float64)
    sin_signed = np.concatenate([-sin[:64], sin[64:]])
    scale = 1.0 / np.sqrt(D)
    ropes = np.stack(
        [cos * scale, sin_signed * scale, cos, sin_signed], axis=1
    ).astype(np.float32)                                   # [128, 4]
    ones = np.ones((D, 1), np.float32)
    onesr = np.ones((1, D), np.float32)
    return ropes, ones, onesr


def kernel(x, past_k, past_v, Wq, Wk, Wv, Wo):
    assert x.shape == (B, 1, HID) and past_k.shape == (B, NKV, S, D)
    x = np.asarray(x, np.float32)
    past_k = np.asarray(past_k, np.float32)
    past_v = np.asarray(past_v, np.float32)
    Wq = np.asarray(Wq, np.float32)
    Wk = np.asarray(Wk, np.float32)
    Wv = np.asarray(Wv, np.float32)
    Wo = np.asarray(Wo, np.float32)

    if "nc" not in _CACHE:
        _CACHE["nc"] = _build_module()
    nc = _CACHE["nc"]

    ropes, ones, onesr = _host_constants()
    xT = np.ascontiguousarray(x[:, 0, :].T)                # [4096, 8]

    in_maps = []
    for c in range(NCORES):
        wq_c = Wq[c * HQ * D:(c + 1) * HQ * D]             # [512, 4096]
        wk_c = Wk[c * D:(c + 1) * D]                       # [128, 4096]
        wv_c = Wv[c * D:(c + 1) * D]
        wqkvT = np.ascontiguousarray(
            np.concatenate([wq_c, wk_c, wv_c], axis=0).T)  # [4096, 768]
        woT = np.ascontiguousarray(Wo[:, c * HQ * D:(c + 1) * HQ * D].T)
        kT_c = np.ascontiguousarray(past_k[:, c].transpose(0, 2, 1))
        v_c = np.ascontiguousarray(past_v[:, c])
        in_maps.append({
            "xT": xT, "wqkvT": wqkvT, "woT": woT, "kT": kT_c, "v": v_c,
            "ropes": ropes, "ones": ones, "onesr": onesr,
        })

    res = run_bass_kernel_spmd(nc, in_maps, list(range(NCORES)))
    acc = np.zeros((B, HID), np.float64)
    for c in range(NCORES):
        acc += res.results[c]["o"]
    return acc.astype(np.float32).reshape(B, 1, HID)



# revision 7
# speedup vs baseline: 3.8740x; 3.8740x over previous
"""Llama3 GQA decode attention (B=8, q_len=1, past=4096) on 8 TRN2 cores.

Sharding: tensor-parallel over heads. Core c owns q-heads [4c, 4c+4) and
kv-head c: Wq/Wk/Wv output-dim sharded, Wo input-dim sharded, KV cache
sharded by kv head. Each core computes a partial o_proj output [8, 4096];
the host sum over cores is the all-reduce.

The kernel is HBM-bandwidth bound (per-core working set = KV cache slice +
weight slices), so all large operands are stored/loaded as bf16 and matmuls
run bf16 x bf16 -> fp32 PSUM. Device-side layouts are partition-major so
every DMA moves >=2KB contiguous per partition (the cost of sub-512B
segments is 2x). Host prepares (data movement + dtype cast only):
  xR    [128, 32, 8]    x.T tiled:   xR[p, t, b] = x[b, t*128+p]
  wR    [128, 32, 768]  wqkvT tiled: wR[p, t, j] = concat(Wq_c,Wk_c,Wv_c)[j, t*128+p]
  woT   [512, 4096]     Wo[:, 512c:512c+512].T
  kT    [8, 128, 4096]  past_k[:, c] with d on partitions
  vR    [8, 128, 32, 128] past_v[:, c] tiled: vR[b, p, t, d] = v[b, t*128+p, d]
All matmuls contract over the partition dim; no large transpose is ever
needed on device.
"""

import sys

sys.path.insert(0, "/opt/trn_rl_repo")

import numpy as np
import ml_dtypes

import concourse.bacc as bacc
import concourse.tile as tile
from concourse import mybir
from concourse.bass_utils import run_bass_kernel_spmd

B = 8            # batch
NH = 32          # query heads total
NKV = 8          # kv heads total
D = 128          # head dim
HID = 4096       # hidden
S = 4096         # past length
NCORES = 8
HQ = NH // NCORES          # 4 query heads per core
QKV = HQ * D + 2 * D       # 768 projection outputs per core
T = S // 128               # 32 seq tiles
ROPE_THETA = 500000.0

F32 = mybir.dt.float32
BF16 = mybir.dt.bfloat16
EXP = mybir.ActivationFunctionType.Exp
BF = ml_dtypes.bfloat16

_CACHE = {}


def _body(nc, tc, ctx, h):
    """Emit one full forward pass. h = dict of dram handles."""
    from contextlib import ExitStack

    consts = ctx.enter_context(tc.tile_pool(name="consts", bufs=1))
    w_pool = ctx.enter_context(tc.tile_pool(name="w", bufs=2))
    kv_pool = ctx.enter_context(tc.tile_pool(name="kv", bufs=3))
    exp_pool = ctx.enter_context(tc.tile_pool(name="exp", bufs=2))
    small = ctx.enter_context(tc.tile_pool(name="small", bufs=2))
    wo_pool = ctx.enter_context(tc.tile_pool(name="wo", bufs=1))
    osb_pool = ctx.enter_context(tc.tile_pool(name="osb", bufs=4))

    # ---- constants / persistent SBUF ----
    x_sb = consts.tile([128, T, B], BF16)
    nc.sync.dma_start(out=x_sb, in_=h["xR"][:, :, :])
    ropes_sb = consts.tile([D, 4], F32)
    nc.sync.dma_start(out=ropes_sb, in_=h["ropes"][:, :])
    ones_sb = consts.tile([D, 1], F32)
    nc.sync.dma_start(out=ones_sb, in_=h["ones"][:, :])
    onesr_sb = consts.tile([1, D], F32)
    nc.sync.dma_start(out=onesr_sb, in_=h["onesr"][:, :])

    # whole projection weight resident in SBUF (bf16, 6.3 MiB)
    w_sb = consts.tile([128, T, QKV], BF16)
    WCH = T // 4
    for ci in range(4):
        sl_ = slice(ci * WCH, (ci + 1) * WCH)
        nc.sync.dma_start(out=w_sb[:, sl_, :], in_=h["wR"][:, sl_, :])

    qT_sb = consts.tile([D, HQ, B], BF16)    # roped qT, scaled by 1/sqrt(D)
    kTn_sb = consts.tile([D, B], BF16)       # roped new-k (kT layout)
    vT_sb = consts.tile([D, B], F32)         # new v, transposed layout
    oT_sb = consts.tile([D, HQ, B], BF16)    # normalized attn out, oT layout

    qcos = ropes_sb[:, 0:1]
    qsin = ropes_sb[:, 1:2]
    kcos = ropes_sb[:, 2:3]
    ksin = ropes_sb[:, 3:4]

    # ---- phase A: QKV projections (weights stationary) ----
    with tc.tile_pool(name="psA", bufs=6, space="PSUM") as psA:
        pj = [psA.tile([D, B], F32, tag="pj", name=f"pj{j}", bufs=6)
              for j in range(HQ + 2)]
        for t in range(T):
            for j in range(HQ + 2):
                nc.tensor.matmul(
                    pj[j], w_sb[:, t, j * D:(j + 1) * D], x_sb[:, t, :],
                    start=(t == 0), stop=(t == T - 1),
                )

        # RoPE on qT (per-partition cos/sin) + scale baked into consts
        for q in range(HQ):
            shuf = small.tile([D, B], F32, tag="shuf")
            nc.vector.tensor_copy(shuf[0:64, :], pj[q][64:128, :])
            nc.vector.tensor_copy(shuf[64:128, :], pj[q][0:64, :])
            nc.vector.tensor_scalar_mul(shuf, shuf, qsin)
            qf = small.tile([D, B], F32, tag="qf")
            nc.vector.scalar_tensor_tensor(
                out=qf, in0=pj[q], scalar=qcos,
                in1=shuf, op0=mybir.AluOpType.mult, op1=mybir.AluOpType.add,
            )
            nc.vector.tensor_copy(qT_sb[:, q, :], qf)    # cast to bf16
        # RoPE on new k
        shufk = small.tile([D, B], F32, tag="shuf")
        nc.vector.tensor_copy(shufk[0:64, :], pj[HQ][64:128, :])
        nc.vector.tensor_copy(shufk[64:128, :], pj[HQ][0:64, :])
        nc.vector.tensor_scalar_mul(shufk, shufk, ksin)
        kf = small.tile([D, B], F32, tag="qf")
        nc.vector.scalar_tensor_tensor(
            out=kf, in0=pj[HQ], scalar=kcos,
            in1=shufk, op0=mybir.AluOpType.mult, op1=mybir.AluOpType.add,
        )
        nc.vector.tensor_copy(kTn_sb, kf)                # cast to bf16
        # new v stays fp32 in transposed layout [d, b]
        nc.vector.tensor_copy(vT_sb, pj[HQ + 1])

    # ---- phase B: attention per batch ----
    with tc.tile_pool(name="psST", bufs=2, space="PSUM") as psST, \
         tc.tile_pool(name="psOT", bufs=2, space="PSUM") as psOT, \
         tc.tile_pool(name="psSL", bufs=1, space="PSUM") as psSL, \
         tc.tile_pool(name="psZ", bufs=1, space="PSUM") as psZ, \
         tc.tile_pool(name="psZB", bufs=1, space="PSUM") as psZB:
        for b in range(B):
            # split cache loads into chunks so multiple DMA queues run in
            # parallel (single-queue BW < HBM peak)
            kt_b = kv_pool.tile([128, T, D], BF16, tag="kt")
            kT_v = h["kT"][b].rearrange("p (t s) -> p t s", s=128)
            v_b = kv_pool.tile([128, T, D], BF16, tag="v")
            CH = T // 4
            for ci in range(4):
                sl_ = slice(ci * CH, (ci + 1) * CH)
                nc.sync.dma_start(out=kt_b[:, sl_, :], in_=kT_v[:, sl_, :])
                nc.sync.dma_start(out=v_b[:, sl_, :], in_=h["vR"][b][:, sl_, :])

            # all 32 score tiles accumulate into one PSUM tile -> one exp
            st = psST.tile([128, T * HQ], F32)
            for t in range(T):
                nc.tensor.matmul(st[:, t * HQ:(t + 1) * HQ], kt_b[:, t, :],
                                 qT_sb[:, :, b], start=True, stop=True)
            exp_sb = exp_pool.tile([128, T, HQ], BF16)
            nc.scalar.activation(
                out=exp_sb.rearrange("p t h -> p (t h)"), in_=st, func=EXP)

            oT_ps = psOT.tile([D, HQ], F32)
            for t in range(T):
                nc.tensor.matmul(oT_ps, v_b[:, t, :], exp_sb[:, t, :],
                                 start=(t == 0), stop=(t == T - 1))
            # current position (no mask needed: q_pos >= all k_pos)
            sl = psSL.tile([1, HQ], F32)
            nc.tensor.matmul(sl, kTn_sb[:, b:b + 1], qT_sb[:, :, b],
                             start=True, stop=True)
            expl = small.tile([1, HQ], F32, tag="expl")
            nc.scalar.activation(out=expl, in_=sl, func=EXP)
            # rank-1 update v_new[d] * expl[h], via broadcast matmul + DVE
            eb_ps = psZB.tile([128, HQ], F32, tag="eb")
            nc.tensor.matmul(eb_ps, onesr_sb, expl, start=True, stop=True)
            vl_sb = small.tile([128, HQ], F32, tag="vl")
            nc.vector.tensor_scalar_mul(vl_sb, eb_ps, vT_sb[:, b:b + 1])

            # softmax denominator Z = sum(exp)  (partition+tile sum)
            zpart = small.tile([128, HQ], F32, tag="zpart")
            nc.vector.reduce_sum(
                out=zpart, in_=exp_sb.rearrange("p t h -> p h t"),
                axis=mybir.AxisListType.X)
            z_ps = psZ.tile([1, HQ], F32)
            nc.tensor.matmul(z_ps, ones_sb, zpart, start=True, stop=False)
            nc.tensor.matmul(z_ps, ones_sb[0:1, :], expl,
                             start=False, stop=True)
            rz = small.tile([1, HQ], F32, tag="rz")
            nc.vector.reciprocal(rz, z_ps)
            zb_ps = psZB.tile([128, HQ], F32, tag="zb")
            nc.tensor.matmul(zb_ps, onesr_sb, rz, start=True, stop=True)
            zb_sb = small.tile([128, HQ], F32, tag="zbs")
            nc.vector.tensor_copy(zb_sb, zb_ps)
            # (cache PV + new-token term), normalize, scatter to [d, h, b]
            s1_sb = small.tile([128, HQ], F32, tag="s1")
            nc.vector.tensor_add(s1_sb, oT_ps, vl_sb)
            nc.vector.tensor_mul(oT_sb[:, :, b], s1_sb, zb_sb)

    # ---- phase C: o_proj partial ----
    with tc.tile_pool(name="psO", bufs=4, space="PSUM") as psO:
        wo_sb = []
        for q in range(HQ):
            w = wo_pool.tile([D, HID], BF16, tag=f"wo{q}")
            nc.sync.dma_start(out=w, in_=h["woT"][q * D:(q + 1) * D, :])
            wo_sb.append(w)
        for n in range(HID // 512):
            o_ps = psO.tile([B, 512], F32)
            for q in range(HQ):
                nc.tensor.matmul(
                    o_ps, oT_sb[:, q, :], wo_sb[q][:, n * 512:(n + 1) * 512],
                    start=(q == 0), stop=(q == HQ - 1))
            o_sb = osb_pool.tile([B, 512], F32)
            nc.vector.tensor_copy(o_sb, o_ps)
            nc.sync.dma_start(out=h["o"][:, n * 512:(n + 1) * 512], in_=o_sb)


def _build_module(reps=1):
    nc = bacc.Bacc()
    h = {
        "xR": nc.declare_dram_parameter("xR", [128, T, B], BF16, isOutput=False),
        "wR": nc.declare_dram_parameter("wR", [128, T, QKV], BF16, isOutput=False),
        "woT": nc.declare_dram_parameter("woT", [HQ * D, HID], BF16, isOutput=False),
        "kT": nc.declare_dram_parameter("kT", [B, D, S], BF16, isOutput=False),
        "vR": nc.declare_dram_parameter("vR", [B, 128, T, D], BF16, isOutput=False),
        "ropes": nc.declare_dram_parameter("ropes", [D, 4], F32, isOutput=False),
        "ones": nc.declare_dram_parameter("ones", [D, 1], F32, isOutput=False),
        "onesr": nc.declare_dram_parameter("onesr", [1, D], F32, isOutput=False),
        "o": nc.declare_dram_parameter("o", [B, HID], F32, isOutput=True),
    }

    with tile.TileContext(nc) as tc:
        from contextlib import ExitStack

        if reps == 1:
            with ExitStack() as ctx:
                _body(nc, tc, ctx, h)
        else:
            with tc.For_i(0, reps, 1):
                with ExitStack() as ctx:
                    _body(nc, tc, ctx, h)

    nc.compile()
    return nc


def _host_constants():
    inv = ROPE_THETA ** (-np.arange(0, 64, dtype=np.float64) * 2.0 / D)
    ang = float(S) * inv
    cos = np.cos(np.concatenate([ang, ang])).astype(np.float64)
    sin = np.sin(np.concatenate([ang, ang])).astype(np.float64)
    sin_signed = np.concatenate([-sin[:64], sin[64:]])
    scale = 1.0 / np.sqrt(D)
    ropes = np.stack(
        [cos * scale, sin_signed * scale, cos, sin_signed], axis=1
    ).astype(np.float32)                                   # [128, 4]
    ones = np.ones((D, 1), np.float32)
    onesr = np.ones((1, D), np.float32)
    return ropes, ones, onesr


def _in_maps(x, past_k, past_v, Wq, Wk, Wv, Wo):
    ropes, ones, onesr = _host_constants()
    # xR[p, t, b] = x[b, t*128+p]
    xR = np.ascontiguousarray(
        x[:, 0, :].T.reshape(T, 128, B).transpose(1, 0, 2)).astype(BF)
    in_maps = []
    for c in range(NCORES):
        wq_c = Wq[c * HQ * D:(c + 1) * HQ * D]             # [512, 4096]
        wk_c = Wk[c * D:(c + 1) * D]                       # [128, 4096]
        wv_c = Wv[c * D:(c + 1) * D]
        wqkvT = np.concatenate([wq_c, wk_c, wv_c], axis=0).T  # [4096, 768]
        wR = np.ascontiguousarray(
            wqkvT.reshape(T, 128, QKV).transpose(1, 0, 2)).astype(BF)
        woT = np.ascontiguousarray(
            Wo[:, c * HQ * D:(c + 1) * HQ * D].T).astype(BF)
        kT_c = np.ascontiguousarray(
            past_k[:, c].transpose(0, 2, 1)).astype(BF)
        # vR[b, p, t, d] = past_v[b, c, t*128+p, d]
        vR = np.ascontiguousarray(
            past_v[:, c].reshape(B, T, 128, D).transpose(0, 2, 1, 3)).astype(BF)
        in_maps.append({
            "xR": xR, "wR": wR, "woT": woT, "kT": kT_c, "vR": vR,
            "ropes": ropes, "ones": ones, "onesr": onesr,
        })
    return in_maps


def kernel(x, past_k, past_v, Wq, Wk, Wv, Wo):
    assert x.shape == (B, 1, HID) and past_k.shape == (B, NKV, S, D)
    x = np.asarray(x, np.float32)
    past_k = np.asarray(past_k, np.float32)
    past_v = np.asarray(past_v, np.float32)
    Wq = np.asarray(Wq, np.float32)
    Wk = np.asarray(Wk, np.float32)
    Wv = np.asarray(Wv, np.float32)
    Wo = np.asarray(Wo, np.float32)

    if "nc" not in _CACHE:
        _CACHE["nc"] = _build_module()
    nc = _CACHE["nc"]

    in_maps = _in_maps(x, past_k, past_v, Wq, Wk, Wv, Wo)
    res = run_bass_kernel_spmd(nc, in_maps, list(range(NCORES)))
    acc = np.zeros((B, HID), np.float64)
    for c in range(NCORES):
        acc += res.results[c]["o"]
    return acc.astype(np.float32).reshape(B, 1, HID)
